# revision 11
# baseline (speedup 1.0000x reference)
"""FAGCN (2-layer FAConv) Trainium2 Bass kernel, 8-core SPMD.

- Nodes sharded 8192/core (dst-shard). Per core, in-edges split into two
  buckets by src half so dma_gather's 15-bit indices address a 32768-row
  window of the node table.
- Per layer a node table T[65536, 128] bf16 is built (row v = [h*dis as 64
  bf16 | al f32 | ar f32 | pad]) and AllGathered across the 8 cores.
- Each bucket is a degree-sorted padded CSR: dst rank r -> (p=r%128,
  g=r//128); group g padded to the max in-bucket degree of its 128 dsts
  (max over cores so one NEFF serves all cores). dma_gather pulls edge rows,
  DVE forms tanh(al+ar)*mask weights, multiplies, and a strided
  tensor_reduce does the segmented sum. dma_scatter_add (distinct indices)
  permutes rank-space partials back to node order in a DRAM accumulator.
- x@W1 prologue and W2/log_softmax epilogue run on PE.
"""
import sys
sys.path.insert(0, "/opt/trn_rl_repo")

import numpy as np

N, F, H, C, E, L = 65536, 512, 64, 40, 1048576, 2
EPS = 0.1
NCORES = 8
SHARD = N // NCORES            # 8192
HALF = N // 2                  # 32768
G128 = SHARD // 128            # 64 groups per shard
ROWW = 128                     # bf16 elems per table row (256B)
MAX_JC = 64                    # max gather cols per chunk
MAX_NG = 16                    # max groups per chunk


def _wrap16_rep(flat_i16):
    """flat idx i -> [16, n/16] with i at (i%16, i//16), replicated to 128
    partitions (8 copies, one per Q7 core)."""
    n = flat_i16.shape[0]
    assert n % 16 == 0
    w = flat_i16.reshape(n // 16, 16).T
    return np.ascontiguousarray(np.tile(w, (8, 1)), dtype=np.int16)


def _plan_chunks(Ks):
    """Greedy-chunk the (descending) common K profile into uniform-K chunks.
    Returns (chunks [(g0, n_g, K, col0)], total_cols)."""
    chunks, col0, g = [], 0, 0
    while g < len(Ks):
        K = int(Ks[g])
        if K == 0:
            break
        n_g = 1
        while (n_g < MAX_NG and g + n_g < len(Ks) and Ks[g + n_g] > 0
               and (n_g + 1) * K <= MAX_JC):
            n_g += 1
        chunks.append((g, n_g, K, col0))
        col0 += n_g * K
        g += n_g
    return chunks, col0


def _preprocess(edge_index):
    src = np.asarray(edge_index[0], dtype=np.int64)
    dst = np.asarray(edge_index[1], dtype=np.int64)
    deg_all = np.bincount(dst, minlength=N).astype(np.float32)

    percore = []
    for c in range(NCORES):
        sel = (dst // SHARD) == c
        s_c, d_c = src[sel], dst[sel] - c * SHARD
        halves = []
        for hf in range(2):
            m = (s_c // HALF) == hf
            ss = (s_c[m] - hf * HALF).astype(np.int64)
            dd = d_c[m]
            k = np.bincount(dd, minlength=SHARD)
            order = np.argsort(-k, kind="stable")       # rank -> dst id
            Kg = k[order].reshape(G128, 128).max(axis=1)
            halves.append(dict(ss=ss, dd=dd, k=k, order=order, Kg=Kg))
        percore.append(halves)

    plans = []
    for hf in range(2):
        Kcom = np.max(np.stack([percore[c][hf]["Kg"] for c in range(NCORES)]),
                      axis=0)
        chunks, Jtot = _plan_chunks(Kcom)
        Gp = sum(n for (_, n, _, _) in chunks)
        plans.append(dict(chunks=chunks, J=Jtot, Gp=Gp))

    cores = []
    for c in range(NCORES):
        buckets = []
        for hf in range(2):
            info, pl = percore[c][hf], plans[hf]
            J, Gp = pl["J"], pl["Gp"]
            order, k = info["order"], info["k"]
            rank_of = np.empty(SHARD, dtype=np.int64)
            rank_of[order] = np.arange(SHARD)
            # group -> (K, colstart) maps for present groups
            colstart = np.full(G128, -1, dtype=np.int64)
            for (g0, n_g, K, col0) in pl["chunks"]:
                for gi in range(n_g):
                    colstart[g0 + gi] = col0 + gi * K
            Kof = np.zeros(G128, dtype=np.int64)
            for (g0, n_g, K, col0) in pl["chunks"]:
                Kof[g0:g0 + n_g] = K
            # vectorized edge fill
            o2 = np.argsort(info["dd"], kind="stable")
            dd_s, ss_s = info["dd"][o2], info["ss"][o2]
            starts = np.zeros(SHARD + 1, dtype=np.int64)
            np.cumsum(k, out=starts[1:])
            j_in = np.arange(len(dd_s)) - starts[dd_s]     # within-dst pos
            r = rank_of[dd_s]
            p_of, g_of = r % 128, r // 128
            col = colstart[g_of] + j_in
            valid = colstart[g_of] >= 0
            assert valid.all() and (j_in < Kof[g_of]).all()
            slot = col * 128 + p_of
            eidx = np.zeros(J * 128, dtype=np.int16)
            eidx[slot] = ss_s.astype(np.int16)
            maskf = np.zeros(J * 128, dtype=np.float32)
            maskf[slot] = 1.0
            mask = maskf.reshape(J, 128).T.copy()          # [128, J]
            # meta / scatter indices over present groups (chunk order)
            midx = np.zeros(Gp * 128, dtype=np.int64)
            sidx = np.zeros(Gp * 128, dtype=np.int16)
            mpos = 0
            for (g0, n_g, K, col0) in pl["chunks"]:
                for gi in range(n_g):
                    rs = (g0 + gi) * 128 + np.arange(128)
                    ds = order[rs]
                    pos = (mpos + gi) * 128 + np.arange(128)
                    midx[pos] = c * SHARD + ds
                    sidx[pos] = ds
                mpos += n_g
            buckets.append(dict(
                eidx=_wrap16_rep(eidx),
                mask=np.ascontiguousarray(mask),
                midx_0=_wrap16_rep(np.clip(midx, 0, HALF - 1).astype(np.int16)),
                midx_1=_wrap16_rep(np.clip(midx - HALF, 0,
                                           HALF - 1).astype(np.int16)),
                sidx=_wrap16_rep(sidx),
            ))
        deg_own = deg_all[c * SHARD:(c + 1) * SHARD].reshape(G128, 128).T
        cores.append(dict(buckets=buckets,
                          deg=np.ascontiguousarray(deg_own, dtype=np.float32)))
    return plans, cores


_BUILD_CACHE = {}


def _build(plans, stage=99, nchunks=999, noscat=0, gonly=0, gsrc="tab"):
    import concourse.bass as bass
    import concourse.mybir as mybir
    import concourse.bacc as bacc
    import concourse.tile as tile
    from concourse import library_config
    from concourse.masks import make_identity

    dt = mybir.dt
    AF = mybir.ActivationFunctionType
    OP = mybir.AluOpType
    nc = bacc.Bacc("TRN2", target_bir_lowering=False, debug=False,
                   num_devices=NCORES)

    # ---------- I/O ----------
    x_t = nc.dram_tensor("x_t", [F, SHARD], dt.float32, kind="ExternalInput")
    W1r = nc.dram_tensor("W1r", [F, H], dt.float32, kind="ExternalInput")
    b1r = nc.dram_tensor("b1r", [128, H], dt.float32, kind="ExternalInput")
    attd = {}
    for l in range(L):
        for nm in ("l", "r"):
            attd[(nm, l)] = nc.dram_tensor(
                f"att{nm}{l}", [128, H], dt.float32, kind="ExternalInput")
    W2r = nc.dram_tensor("W2r", [H, C], dt.float32, kind="ExternalInput")
    b2r = nc.dram_tensor("b2r", [128, C], dt.float32, kind="ExternalInput")
    degf = nc.dram_tensor("degf", [128, G128], dt.float32, kind="ExternalInput")
    sel_d = nc.dram_tensor("sel", [128, 1], dt.float32, kind="ExternalInput")
    zeros_acc = nc.dram_tensor("zeros_acc", [SHARD, H], dt.float32,
                               kind="ExternalInput")
    eidx_d, mask_d, midx_d, sidx_d = {}, {}, {}, {}
    for hf in range(2):
        J, Gp = plans[hf]["J"], plans[hf]["Gp"]
        eidx_d[hf] = nc.dram_tensor(f"eidx{hf}", [128, J * 8], dt.int16,
                                    kind="ExternalInput")
        mask_d[hf] = nc.dram_tensor(f"mask{hf}", [128, J], dt.float32,
                                    kind="ExternalInput")
        for w in range(2):
            midx_d[(hf, w)] = nc.dram_tensor(
                f"midx{hf}_{w}", [128, Gp * 8], dt.int16, kind="ExternalInput")
        sidx_d[hf] = nc.dram_tensor(f"sidx{hf}", [128, Gp * 8], dt.int16,
                                    kind="ExternalInput")
    out = nc.dram_tensor("out", [SHARD, C], dt.float32, kind="ExternalOutput")

    tab_own = nc.dram_tensor("tab_own", [SHARD, ROWW], dt.bfloat16,
                             kind="Internal")
    import os as _os
    _tshared = _os.environ.get("FAGCN_SHARED", "1") == "1"
    table = nc.dram_tensor("table", [N, ROWW], dt.bfloat16, kind="Internal",
                           addr_space="Shared" if _tshared else "Local")
    h_accum = nc.dram_tensor("h_accum", [SHARD, H], dt.float32,
                             kind="Internal")
    dumtab = (nc.dram_tensor("dumtab", [N, ROWW], dt.bfloat16,
                             kind="ExternalInput") if gsrc == "ext" else None)

    JMAX = max(plans[0]["J"], plans[1]["J"])
    GPMAX = max(plans[0]["Gp"], plans[1]["Gp"])

    with tile.TileContext(nc) as tc:
        with tc.tile_pool(name="pp", bufs=1) as pp, \
             tc.tile_pool(name="b1p", bufs=1) as b1p, \
             tc.tile_pool(name="wp", bufs=2) as wp, \
             tc.tile_pool(name="psp", bufs=2, space="PSUM") as psp:
            nc.gpsimd.load_library(library_config.mlp)

            # persistent
            h0 = pp.tile([128, G128, H], dt.float32)
            h = pp.tile([128, G128, H], dt.float32)
            dis = pp.tile([128, G128], dt.float32)
            selt = pp.tile([128, 1], dt.float32)
            b1t = pp.tile([128, H], dt.float32)
            b2t = pp.tile([128, C], dt.float32)
            lg = pp.tile([128, G128, C], dt.float32)
            attt = {}
            for kk in attd:
                attt[kk] = pp.tile([128, H], dt.float32, name=f"att{kk[0]}{kk[1]}t",
                                   tag=f"att{kk[0]}{kk[1]}")
                nc.sync.dma_start(attt[kk][:], attd[kk][:])
            W1t = pp.tile([128, 4, H], dt.float32)
            nc.sync.dma_start(W1t[:],
                              W1r[:].rearrange("(c p) h -> p c h", p=128))
            W2t = pp.tile([H, C], dt.float32)
            nc.sync.dma_start(W2t[:], W2r[:])
            nc.sync.dma_start(b1t[:], b1r[:])
            nc.sync.dma_start(b2t[:], b2r[:])
            nc.sync.dma_start(selt[:], sel_d[:])
            ident = pp.tile([128, 128], dt.float32)
            make_identity(nc, ident[:])

            eidx_t, mask_t, midx_t, sidx_t = {}, {}, {}, {}
            for hf in range(2):
                J, Gp = plans[hf]["J"], plans[hf]["Gp"]
                eidx_t[hf] = pp.tile([128, J * 8], dt.int16, name=f"ei{hf}t", tag=f"ei{hf}")
                nc.sync.dma_start(eidx_t[hf][:], eidx_d[hf][:])
                mask_t[hf] = pp.tile([128, J], dt.float32, name=f"mk{hf}t", tag=f"mk{hf}")
                nc.sync.dma_start(mask_t[hf][:], mask_d[hf][:])
                for w in range(2):
                    midx_t[(hf, w)] = pp.tile([128, Gp * 8], dt.int16,
                                              name=f"mi{hf}{w}t",
                                              tag=f"mi{hf}{w}")
                    nc.sync.dma_start(midx_t[(hf, w)][:], midx_d[(hf, w)][:])
                sidx_t[hf] = pp.tile([128, Gp * 8], dt.int16, name=f"si{hf}t", tag=f"si{hf}")
                nc.sync.dma_start(sidx_t[hf][:], sidx_d[hf][:])

            # dis = (deg > 0) / sqrt(max(deg, 1))
            degt = pp.tile([128, G128], dt.float32)
            nc.sync.dma_start(degt[:], degf[:])
            t_a = pp.tile([128, G128], dt.float32)
            t_b = pp.tile([128, G128], dt.float32)
            nc.vector.tensor_scalar(out=t_a[:], in0=degt[:], scalar1=1.0,
                                    scalar2=None, op0=OP.max)
            nc.scalar.activation(t_a[:], t_a[:], AF.Sqrt)
            nc.vector.reciprocal(t_b[:], t_a[:])
            nc.vector.tensor_scalar(out=t_a[:], in0=degt[:], scalar1=0.0,
                                    scalar2=None, op0=OP.is_gt)
            nc.vector.tensor_tensor(out=dis[:], in0=t_b[:], in1=t_a[:],
                                    op=OP.mult)

            # prologue: h0 = relu(x @ W1 + b1)
            for t in range(G128):
                ps = psp.tile([128, H], dt.float32, tag="mmps")
                for fc in range(4):
                    xt = wp.tile([128, 128], dt.float32, tag="xt")
                    nc.sync.dma_start(
                        xt[:], x_t[fc * 128:(fc + 1) * 128,
                                   t * 128:(t + 1) * 128])
                    nc.tensor.matmul(ps[:], xt[:], W1t[:, fc, :],
                                     start=(fc == 0), stop=(fc == 3))
                hb = wp.tile([128, H], dt.float32, tag="hb")
                nc.vector.tensor_tensor(out=hb[:], in0=ps[:], in1=b1t[:],
                                        op=OP.add)
                nc.scalar.activation(h0[:, t, :], hb[:], AF.Relu)
            nc.vector.tensor_copy(h[:], h0[:])

            scr = b1p.tile([128, G128, H], dt.float32)   # shared scratch
            alt_dummy = b1p.tile([128, MAX_NG], dt.bfloat16)
            alt_dummy2 = b1p.tile([128, MAX_JC], dt.bfloat16)
            zt = b1p.tile([128, G128, H], dt.float32)
            nc.vector.memset(zt[:], 0.0)
            for l in range(L):
                if stage < 2:
                    break
                # table build: row = [h*dis bf16 | al f32 | ar f32 | pad]
                tab = b1p.tile([128, G128, ROWW], dt.bfloat16, tag="tab")
                tabf = tab[:].bitcast(dt.float32)
                nc.vector.tensor_tensor(
                    out=tab[:, :, 0:H], in0=h[:],
                    in1=dis[:].unsqueeze(2).broadcast_to([128, G128, H]),
                    op=OP.mult)
                alt = wp.tile([128, G128], dt.float32, tag="alt")
                for nm, fcol in (("l", 32), ("r", 33)):
                    nc.vector.tensor_tensor(
                        out=scr[:], in0=h[:],
                        in1=attt[(nm, l)][:].unsqueeze(1).broadcast_to(
                            [128, G128, H]),
                        op=OP.mult)
                    nc.vector.tensor_reduce(out=alt[:], in_=scr[:],
                                            axis=mybir.AxisListType.X,
                                            op=OP.add)
                    nc.vector.tensor_copy(
                        tabf[:, :, fcol:fcol + 1].squeeze(2), alt[:])
                nc.sync.dma_start(
                    tab_own[:].rearrange("(g p) f -> p g f", p=128), tab[:])
                nc.gpsimd.collective_compute(
                    "AllGather", OP.bypass,
                    replica_groups=[list(range(NCORES))],
                    ins=[tab_own[:]], outs=[table[:]])
                nc.sync.dma_start(
                    h_accum[:].rearrange("(g p) f -> p g f", p=128), zt[:])

                for hf in (range(2) if stage >= 4 else []):
                    pl = plans[hf]
                    gt = dumtab if gsrc == "ext" else table
                    win = gt[hf * HALF:(hf + 1) * HALF, :]
                    mpos = 0
                    for ci, (g0, n_g, K, col0) in enumerate(pl["chunks"]):
                        if ci >= nchunks:
                            break
                        Jc = n_g * K
                        msg = wp.tile([128, MAX_JC, ROWW], dt.bfloat16,
                                      tag="msg")
                        nc.gpsimd.dma_gather(
                            out_ap=msg[:, 0:Jc, :], in_ap=win,
                            idxs_ap=eidx_t[hf][:, col0 * 8:(col0 + Jc) * 8],
                            num_idxs=Jc * 128, num_idxs_reg=Jc * 128,
                            elem_size=ROWW, single_packet=False)
                        meta = {}
                        for w in range(2):
                            meta[w] = wp.tile([128, MAX_NG, ROWW],
                                              dt.bfloat16, name=f"meta{w}t",
                                              tag=f"meta{w}")
                            nc.gpsimd.dma_gather(
                                out_ap=meta[w][:, 0:n_g, :],
                                in_ap=gt[w * HALF:(w + 1) * HALF, :],
                                idxs_ap=midx_t[(hf, w)][
                                    :, mpos * 8:(mpos + n_g) * 8],
                                num_idxs=n_g * 128, num_idxs_reg=n_g * 128,
                                elem_size=ROWW, single_packet=False)
                        if gonly:
                            nc.vector.tensor_copy(
                                alt_dummy[:, 0:n_g],
                                meta[0][:, 0:n_g, 0:1].squeeze(2))
                            nc.vector.tensor_copy(
                                alt_dummy2[:, 0:Jc],
                                msg[:, 0:Jc, 0:1].squeeze(2))
                            mpos += n_g
                            continue
                        m0f = meta[0][:].bitcast(dt.float32)
                        m1f = meta[1][:].bitcast(dt.float32)
                        ar_t = wp.tile([128, MAX_NG], dt.float32, tag="ar_t")
                        nc.vector.tensor_tensor(
                            out=ar_t[:, 0:n_g],
                            in0=m0f[:, 0:n_g, 33:34].squeeze(2),
                            in1=m1f[:, 0:n_g, 33:34].squeeze(2),
                            op=OP.subtract)
                        nc.vector.scalar_tensor_tensor(
                            out=ar_t[:, 0:n_g], in0=ar_t[:, 0:n_g],
                            scalar=selt[:, 0:1],
                            in1=m1f[:, 0:n_g, 33:34].squeeze(2),
                            op0=OP.mult, op1=OP.add)
                        msgf = msg[:].bitcast(dt.float32)
                        wfin = wp.tile([128, MAX_JC], dt.float32, tag="wfin")
                        nc.vector.tensor_tensor(
                            out=wfin[:, 0:Jc].rearrange(
                                "p (g k) -> p g k", g=n_g),
                            in0=msgf[:, 0:Jc, 32:33].squeeze(2).rearrange(
                                "p (g k) -> p g k", g=n_g),
                            in1=ar_t[:, 0:n_g].unsqueeze(2).broadcast_to(
                                [128, n_g, K]),
                            op=OP.add)
                        nc.scalar.activation(wfin[:, 0:Jc], wfin[:, 0:Jc],
                                             AF.Tanh)
                        nc.vector.tensor_tensor(
                            out=wfin[:, 0:Jc], in0=wfin[:, 0:Jc],
                            in1=mask_t[hf][:, col0:col0 + Jc], op=OP.mult)
                        prod = wp.tile([128, MAX_JC, H], dt.bfloat16,
                                       tag="prod")
                        nc.vector.tensor_tensor(
                            out=prod[:, 0:Jc, :], in0=msg[:, 0:Jc, 0:H],
                            in1=wfin[:, 0:Jc].unsqueeze(2).broadcast_to(
                                [128, Jc, H]),
                            op=OP.mult)
                        part = wp.tile([128, MAX_NG, H], dt.float32,
                                       tag="part")
                        nc.vector.tensor_reduce(
                            out=part[:, 0:n_g, :],
                            in_=prod[:, 0:Jc, :].rearrange(
                                "p (g k) f -> p g f k", g=n_g),
                            axis=mybir.AxisListType.X, op=OP.add)
                        if not noscat:
                            nc.gpsimd.dma_scatter_add(
                                out_ap=h_accum[:], in_ap=part[:, 0:n_g, :],
                                idxs_ap=sidx_t[hf][:, mpos * 8:(mpos + n_g) * 8],
                                num_idxs=n_g * 128, num_idxs_reg=n_g * 128,
                                elem_size=H, single_packet=False)
                        mpos += n_g

                # epilogue: h = dis * hsum + EPS * h0
                if stage < 3:
                    continue
                nc.sync.dma_start(
                    scr[:], h_accum[:].rearrange("(g p) f -> p g f", p=128))
                nc.vector.tensor_tensor(
                    out=scr[:], in0=scr[:],
                    in1=dis[:].unsqueeze(2).broadcast_to([128, G128, H]),
                    op=OP.mult)
                nc.vector.scalar_tensor_tensor(
                    out=h[:], in0=h0[:], scalar=EPS, in1=scr[:],
                    op0=OP.mult, op1=OP.add)

            # final: logits + log_softmax
            for t in range(G128):
                pst = psp.tile([H, 128], dt.float32, tag="pst")
                nc.tensor.transpose(out=pst[:], in_=h[:, t, :],
                                    identity=ident[:])
                hT = wp.tile([H, 128], dt.float32, tag="hT")
                nc.scalar.activation(hT[:], pst[:], AF.Copy)
                ps2 = psp.tile([128, C], dt.float32, tag="ps2")
                nc.tensor.matmul(ps2[:], hT[:], W2t[:], start=True, stop=True)
                nc.vector.tensor_tensor(out=lg[:, t, :], in0=ps2[:],
                                        in1=b2t[:], op=OP.add)
            mx = wp.tile([128, G128], dt.float32, tag="mx")
            nc.vector.tensor_reduce(out=mx[:], in_=lg[:],
                                    axis=mybir.AxisListType.X, op=OP.max)
            nc.vector.tensor_tensor(
                out=lg[:], in0=lg[:],
                in1=mx[:].unsqueeze(2).broadcast_to([128, G128, C]),
                op=OP.subtract)
            exs = scr[:, :, 0:C]
            nc.scalar.activation(exs, lg[:], AF.Exp)
            sm = wp.tile([128, G128], dt.float32, tag="sm")
            nc.vector.tensor_reduce(out=sm[:], in_=exs,
                                    axis=mybir.AxisListType.X, op=OP.add)
            nc.scalar.activation(sm[:], sm[:], AF.Ln)
            nc.vector.tensor_tensor(
                out=lg[:], in0=lg[:],
                in1=sm[:].unsqueeze(2).broadcast_to([128, G128, C]),
                op=OP.subtract)
            nc.sync.dma_start(out[:].rearrange("(g p) c -> p g c", p=128),
                              lg[:])

    nc.compile()
    return nc


def _run(inputs, trace=False):
    from concourse.bass_utils import run_bass_kernel_spmd

    x = np.asarray(inputs["x"], dtype=np.float32)
    edge_index = np.asarray(inputs["edge_index"])
    W1 = np.asarray(inputs["W1"], dtype=np.float32)
    b1 = np.asarray(inputs["b1"], dtype=np.float32)
    att_l = np.asarray(inputs["att_l"], dtype=np.float32)
    att_r = np.asarray(inputs["att_r"], dtype=np.float32)
    W2 = np.asarray(inputs["W2"], dtype=np.float32)
    b2 = np.asarray(inputs["b2"], dtype=np.float32)

    plans, cores = _preprocess(edge_index)
    key = tuple(tuple(plans[hf]["chunks"]) for hf in range(2))
    import os
    stage = int(os.environ.get("FAGCN_STAGE", "99"))
    nchunks = int(os.environ.get("FAGCN_NCHUNKS", "999"))
    noscat = int(os.environ.get("FAGCN_NOSCAT", "0"))
    gonly = int(os.environ.get("FAGCN_GONLY", "0"))
    gsrc = os.environ.get("FAGCN_GSRC", "tab")
    key = (key, stage, nchunks, noscat, gonly, gsrc, os.environ.get("FAGCN_SHARED", "1"))
    if key not in _BUILD_CACHE:
        _BUILD_CACHE[key] = _build(plans, stage, nchunks, noscat, gonly, gsrc)
    nc = _BUILD_CACHE[key]

    x_t = np.ascontiguousarray(x.T)
    zeros = np.zeros((SHARD, H), np.float32)
    global _DUMTAB
    if os.environ.get("FAGCN_GSRC", "tab") == "ext":
        import jax.numpy as jnp
        _DUMTAB = np.asarray(jnp.zeros((N, ROWW), dtype=jnp.bfloat16))
    in_maps = []
    for c in range(NCORES):
        cc = cores[c]
        m = dict(
            x_t=np.ascontiguousarray(x_t[:, c * SHARD:(c + 1) * SHARD]),
            W1r=W1, b1r=np.tile(b1[None, :], (128, 1)),
            W2r=W2, b2r=np.tile(b2[None, :], (128, 1)),
            degf=cc["deg"],
            sel=np.full((128, 1), 1.0 if c < 4 else 0.0, np.float32),
            zeros_acc=zeros,
        )
        if os.environ.get("FAGCN_GSRC", "tab") == "ext":
            m["dumtab"] = _DUMTAB
        for l in range(L):
            m[f"attl{l}"] = np.tile(att_l[l][None, :], (128, 1))
            m[f"attr{l}"] = np.tile(att_r[l][None, :], (128, 1))
        for hf in range(2):
            bk = cc["buckets"][hf]
            m[f"eidx{hf}"] = bk["eidx"]
            m[f"mask{hf}"] = bk["mask"]
            m[f"midx{hf}_0"] = bk["midx_0"]
            m[f"midx{hf}_1"] = bk["midx_1"]
            m[f"sidx{hf}"] = bk["sidx"]
        in_maps.append(m)

    res = run_bass_kernel_spmd(nc, in_maps, core_ids=list(range(NCORES)),
                               trace=trace)
    outs = [res.results[c]["out"] for c in range(NCORES)]
    return np.concatenate(outs, axis=0), res


def kernel(**inputs):
    out, _ = _run(inputs, trace=False)
    return out
